# revision 9
# baseline (speedup 1.0000x reference)
"""Bass/Trainium2 kernel for nn_BinaryLSTMCell (B=65536, D=U=256).

Data-parallel over 8 cores (8192 rows each). Per-core dataflow:
  - x and h ship as fp8(e4m3) hi+lo pairs; binarized +-1 weights are
    exact in fp8, so the x/r projections run as DoubleRow matmuls
    (K=256 per instruction, 0.5 cyc/row) - half the PE cycles of f32r.
  - Per 128-row tile: x-matmuls -> PSUM, inner hard-tanh clamps the
    PSUM in place, r-matmuls accumulate on top, ACT evicts pre-gates
    to bf16, Pool clamps the gates, DVE runs the elementwise tail.
  - The inner clamp is load-balanced: most tiles clamp on DVE (fused
    tensor_scalar); a pattern of tiles (KPAT 'C') runs it on ACT as a
    2-op Relu chain producing 1-clamp(xp), accumulates NEGATED
    r-weights, and folds the sign/offset back in the eviction's
    Copy(scale=-1, bias=1) at zero extra cost.
  - Scheduling: PE warm-up matmuls ramp the p-state during the DMA
    fill; tails are deferred into the next super-tile at two insertion
    points; the last super-tile's tail is emitted in quarters as its
    gate tiles become ready.
"""

import os
import sys

for _p in ("/opt/trn_rl_repo", "/root/.axon_site/_ro/trn_rl_repo"):
    if os.path.isdir(_p) and _p not in sys.path:
        sys.path.append(_p)

import numpy as np
from contextlib import ExitStack

import concourse.bass as bass
import concourse.bacc as bacc
import concourse.mybir as mybir
from concourse.tile import TileContext
from concourse.bass_utils import run_bass_kernel_spmd

F32 = mybir.dt.float32
BF16 = mybir.dt.bfloat16
FP8 = mybir.dt.float8e4
ALU = mybir.AluOpType
PM = mybir.MatmulPerfMode

N_CORES = 8
B = 65536
D = 256
U = 256
B_CORE = B // N_CORES          # 8192
SUPER = int(os.environ.get("KSUPER", "1024"))  # batch rows per super-tile
T_PER_S = SUPER // 128         # 8 tiles of 128 rows per super-tile
N_SUPER = B_CORE // SUPER      # 8 super-tiles per core

GATE_DT = BF16


def _clamp(eng, out_ap, in_ap):
    """out = min(max(in, -1), 1) as one fused tensor_scalar."""
    eng.tensor_scalar(out_ap, in_ap, -1.0, 1.0, ALU.max, ALU.min)


def build_program(n_super=N_SUPER, gate_dt=GATE_DT):
    """Per-core SPMD program.

    DRAM layouts (per-core):
      x, h   : [n_super, 128, 2, 2048] fp8; [:, :, hl, k*1024 + t*128 + q]
               holds component hl (0=hi, 1=lo) of element
               [d = k*128 + p, batch row = s*1024 + 8*q + t]
      c      : [n_super, 128, 2048] bf16; free t*256 + u = c[s*1024+8q+t, u]
      wx, wr : [128, 2, 1024] fp8 binarized weights; [p, k, col]
      hn, cn : like c (outputs, bf16)
    """
    nc = bacc.Bacc("TRN2", target_bir_lowering=False, debug=False)

    _c2 = nc.alloc_sbuf_tensor("const-float32-2x0", [128, 1], F32)
    nc.gpsimd.memset(_c2.ap(), 2.0)
    nc.const_aps.aps[(F32, 2.0)] = _c2.ap()
    nc.all_engine_barrier()

    x_d = nc.dram_tensor("x", [n_super, 128, T_PER_S, 2, 2, 128], FP8, kind="ExternalInput")
    h_d = nc.dram_tensor("h", [n_super, 128, T_PER_S, 2, 2, 128], FP8, kind="ExternalInput")
    c_d = nc.dram_tensor("c", [n_super, 128, T_PER_S * 256], BF16, kind="ExternalInput")
    wx_d = nc.dram_tensor("wx", [128, 2, 1024], FP8, kind="ExternalInput")
    wr_d = nc.dram_tensor("wr", [128, 2, 1024], FP8, kind="ExternalInput")
    wrn_d = nc.dram_tensor("wrn", [128, 2, 1024], FP8, kind="ExternalInput")
    id_d = nc.dram_tensor("ident", [128, 128], BF16, kind="ExternalInput")
    hn_d = nc.dram_tensor("hn", [n_super, 128, T_PER_S * 256], BF16, kind="ExternalOutput")
    cn_d = nc.dram_tensor("cn", [n_super, 128, T_PER_S * 256], BF16, kind="ExternalOutput")

    with TileContext(nc) as tc, ExitStack() as ctx:
        wpool = ctx.enter_context(tc.tile_pool(name="w", bufs=1))
        iopool = ctx.enter_context(tc.tile_pool(name="io", bufs=2))
        inpool = ctx.enter_context(tc.tile_pool(name="inp", bufs=int(os.environ.get("KINBUFS", "3"))))
        mpool = ctx.enter_context(tc.tile_pool(name="mid", bufs=int(os.environ.get("KMIDBUFS", "3"))))
        pspool = ctx.enter_context(tc.tile_pool(name="ps", bufs=4, space="PSUM"))
        rtpool = ctx.enter_context(tc.tile_pool(name="rt", bufs=int(os.environ.get("KRTBUFS", "3"))))

        # startup order: wx + x0 first so the first x-matmuls (and the DVE
        # clamps behind them) start as early as possible; wr/wrn arrive
        # during the first tile's x-matmuls.
        wx = wpool.tile([128, 2, 1024], FP8, tag="wx")
        nc.sync.dma_start(wx[:], wx_d.ap()[:, :, :])
        xt0 = inpool.tile([128, T_PER_S, 2, 2, 128], FP8, tag="xt", name="xt_0")
        nc.sync.dma_start(xt0[:, 0:2], x_d.ap()[0][:, 0:2])
        ht0 = inpool.tile([128, T_PER_S, 2, 2, 128], FP8, tag="ht", name="ht_0")
        nc.sync.dma_start(ht0[:, 0:2], h_d.ap()[0][:, 0:2])
        wr = wpool.tile([128, 2, 1024], FP8, tag="wr")
        nc.sync.dma_start(wr[:], wr_d.ap()[:, :, :])
        nc.sync.dma_start(xt0[:, 2:4], x_d.ap()[0][:, 2:4])
        nc.sync.dma_start(ht0[:, 2:4], h_d.ap()[0][:, 2:4])
        wrn = wpool.tile([128, 2, 1024], FP8, tag="wrn")
        nc.sync.dma_start(wrn[:], wrn_d.ap()[:, :, :])
        nc.sync.dma_start(xt0[:, 4:T_PER_S], x_d.ap()[0][:, 4:T_PER_S])
        nc.sync.dma_start(ht0[:, 4:T_PER_S], h_d.ap()[0][:, 4:T_PER_S])
        cc0 = mpool.tile([128, T_PER_S * 256], gate_dt, tag="cc", name="cc_0")
        nc.sync.dma_start(cc0[:], c_d.ap()[0])
        ident = wpool.tile([128, 128], BF16, tag="ident")
        nc.sync.dma_start(ident[:], id_d.ap()[:, :])

        # optional PE p-state warm-up: dummy matmuls on memset data keep the
        # tensor engine busy during the initial DMA fill so real matmuls hit
        # full clock sooner.
        kwarm = int(os.environ.get("KWARM", "0"))
        if kwarm:
            wa = wpool.tile([128, 2, 128], FP8, tag="warm_a")
            wb = wpool.tile([128, 2, 512], FP8, tag="warm_b")
            nc.gpsimd.memset(wa[:], 0.0)
            nc.gpsimd.memset(wb[:], 0.0)

        def stage_in(s):
            if s == 0:
                return xt0, ht0, cc0
            xt = inpool.tile([128, T_PER_S, 2, 2, 128], FP8, tag="xt", name=f"xt_{s}")
            nc.sync.dma_start(xt[:], x_d.ap()[s])
            ht = inpool.tile([128, T_PER_S, 2, 2, 128], FP8, tag="ht", name=f"ht_{s}")
            nc.sync.dma_start(ht[:], h_d.ap()[s])
            cc = mpool.tile([128, T_PER_S * 256], gate_dt, tag="cc", name=f"cc_{s}")
            nc.sync.dma_start(cc[:], c_d.ap()[s])
            return xt, ht, cc

        def dr_slice(t3, hl, t):
            # [128, 8, 2, 2, 128] tile -> [128, 2, 128] stationary AP
            # (k-tiles of the DR matmul) for tile t, component hl.
            return t3[:, t, hl]

        staged = {}
        pending_tail = {}
        KDEFT = int(os.environ.get("KDEFT", "3"))
        KDEFT2 = int(os.environ.get("KDEFT2", "8"))
        KHALVES = int(os.environ.get("KHALVES", "2"))
        for s in range(n_super):
            if s not in staged:
                staged[s] = stage_in(s)
            xt, ht, cc = staged.pop(s)

            gates = mpool.tile([128, T_PER_S, 1024], gate_dt, tag="gates")

            def emit_xproj(t, ps):
                for n in range(2):
                    o = ps[:, n * 512:(n + 1) * 512]
                    w = wx[:, :, n * 512:(n + 1) * 512]
                    nc.tensor.matmul(o, dr_slice(xt, 0, t), w,
                                     start=True, stop=False, perf_mode=PM.DoubleRow)
                    nc.tensor.matmul(o, dr_slice(xt, 1, t), w,
                                     start=False, stop=True, perf_mode=PM.DoubleRow)

            def emit_rproj(t, ps):
                for n in range(2):
                    o = ps[:, n * 512:(n + 1) * 512]
                    w = wr[:, :, n * 512:(n + 1) * 512]
                    nc.tensor.matmul(o, dr_slice(ht, 0, t), w,
                                     start=False, stop=False, perf_mode=PM.DoubleRow,
                                     skip_group_check=True)
                    nc.tensor.matmul(o, dr_slice(ht, 1, t), w,
                                     start=False, stop=(n == 1), perf_mode=PM.DoubleRow,
                                     skip_group_check=True)

            def emit_rproj_neg(t, ps):
                for n in range(2):
                    o = ps[:, n * 512:(n + 1) * 512]
                    w = wrn[:, :, n * 512:(n + 1) * 512]
                    nc.tensor.matmul(o, dr_slice(ht, 0, t), w,
                                     start=False, stop=False, perf_mode=PM.DoubleRow,
                                     skip_group_check=True)
                    nc.tensor.matmul(o, dr_slice(ht, 1, t), w,
                                     start=False, stop=(n == 1), perf_mode=PM.DoubleRow,
                                     skip_group_check=True)

            # software pipeline: tile t+1's x-matmuls overlap tile t's
            # inner clamp + eviction. KCHAIN>0 moves every KCHAIN-th tile's
            # inner hard-tanh onto ACT as a 2-op Relu chain producing
            # 1-clamp(xp); those tiles accumulate NEGATED r-weights so PSUM
            # holds 1-pregate, and the eviction's Copy(scale=-1, bias=1)
            # restores the raw pregate at zero extra cost.
            def g3(lo, hi, gates=gates):
                return gates[:, :, lo:hi]

            def s3(tile):
                return tile[:].rearrange("p (t u) -> p t u", u=256)

            t1 = mpool.tile([128, T_PER_S * 256], gate_dt, tag="t1", name=f"t1_{s}")
            t2 = mpool.tile([128, T_PER_S * 256], gate_dt, tag="t2", name=f"t2_{s}")
            z = mpool.tile([128, T_PER_S * 256], gate_dt, tag="z", name=f"z_{s}")
            cnew = iopool.tile([128, T_PER_S * 256], BF16, tag="cn", name=f"cn_{s}")
            hnew = iopool.tile([128, T_PER_S * 256], BF16, tag="hn", name=f"hn_{s}")

            def emit_tail_part(tlo, thi, s=s, gates=gates, cc=cc, t1=t1, t2=t2,
                               z=z, cnew=cnew, hnew=hnew, g3=g3, s3=s3):
                tteng = nc.gpsimd if os.environ.get("KTT", "dve") == "pool" else nc.vector
                zeng = nc.vector if os.environ.get("KZ", "dve") == "dve" else nc.gpsimd
                t2eng = nc.gpsimd if os.environ.get("KT2", "dve") == "pool" else tteng
                ts_ = slice(tlo, thi)
                cs = slice(tlo * 256, thi * 256)
                tteng.tensor_tensor(s3(t1)[:, ts_], g3(0, 256)[:, ts_], s3(cc)[:, ts_], ALU.mult)
                t2eng.tensor_tensor(s3(t2)[:, ts_], g3(256, 512)[:, ts_], g3(512, 768)[:, ts_], ALU.mult)
                nc.vector.tensor_tensor(s3(cnew)[:, ts_], s3(t1)[:, ts_], s3(t2)[:, ts_], ALU.add)
                nc.sync.dma_start(cn_d.ap()[s][:, cs], cnew[:, cs])
                _clamp(zeng, z[:, cs], cnew[:, cs])
                # o, z in [-1,1] so the outer hard_tanh is the identity
                nc.vector.tensor_tensor(s3(hnew)[:, ts_], g3(768, 1024)[:, ts_], s3(z)[:, ts_], ALU.mult)
                nc.sync.dma_start(hn_d.ap()[s][:, cs], hnew[:, cs])

            kgc = os.environ.get("KGCLAMP", "pool")
            # per-tile inner-hard-tanh mode pattern (len-8 string, one char
            # per tile of the super):
            #   D: DVE tensor_scalar clamp in PSUM, r accumulates on top
            #   C: ACT 2-op Relu chain in PSUM (1-clamp), negated r-weights
            #   A: ACT raw-evict to bf16, DVE 4x clamp, PE identity re-inject
            #   R: DMA raw-evict to f32 SBUF, DVE 2x fused clamp, PE re-inject
            kpat = os.environ.get("KPAT", "DDDDDDDD")
            if s == 0:
                kpat = os.environ.get("KPAT0", kpat)
            elif s == n_super - 1:
                kpat = os.environ.get("KPATL", kpat)
            AF = mybir.ActivationFunctionType
            last = s == n_super - 1
            pss = {}
            mode = {}
            xcs = {}
            # fix-clamp engine pattern: which tiles' gate clamps go to DVE
            kfix = os.environ.get("KFIX", "")
            for g in range(T_PER_S + 1):
                if g < T_PER_S:
                    mode[g] = kpat[g % len(kpat)]
                    pss[g] = pspool.tile([128, 1024], F32, tag="ps",
                                         name=f"ps_{s}_{g}")
                    if s == 0 and g == 0 and kwarm:
                        for _ in range(kwarm):
                            nc.tensor.matmul(pss[g][:, 0:512], wa[:], wb[:],
                                             start=True, stop=True,
                                             perf_mode=PM.DoubleRow,
                                             skip_group_check=True)
                    emit_xproj(g, pss[g])
                    m = mode[g]
                    if m == "C":
                        p = pss[g][:]
                        nc.scalar.activation(p, p, AF.Relu, bias=1.0, scale=1.0)
                        nc.scalar.activation(p, p, AF.Relu, bias=2.0, scale=-1.0)
                    elif m == "A":
                        # raw-evict xp, clamp cheaply in bf16 at 4x, and
                        # re-inject into a SEPARATE r-psum via PE identity
                        # matmul -- the x-psum frees right after the evict
                        # and the r-matmuls never wait on the inner clamp
                        xr = rtpool.tile([128, 1024], gate_dt, tag="xr",
                                         name=f"xr_{s}_{g}")
                        nc.scalar.copy(xr[:], pss[g][:])
                        xc = rtpool.tile([128, 1024], gate_dt, tag="xc",
                                         name=f"xc_{s}_{g}")
                        _clamp(nc.vector, xc[:], xr[:])
                        ps2 = pspool.tile([128, 1024], F32, tag="ps",
                                          name=f"ps2_{s}_{g}")
                        xcs[g] = (xc, ps2)
                    elif m == "R":
                        sx = rtpool.tile([128, 1024], F32, tag="sx",
                                         name=f"sx_{s}_{g}")
                        nc.sync.dma_start(sx[:], pss[g][:])
                        xc = rtpool.tile([128, 1024], gate_dt, tag="xc",
                                         name=f"xc_{s}_{g}")
                        _clamp(nc.vector, xc[:], sx[:])
                        xcs[g] = xc
                    else:
                        _clamp(nc.vector, pss[g][:], pss[g][:])
                if g >= 1:
                    gp = g - 1
                    m = mode[gp]
                    if m == "A":
                        xc, ps2 = xcs.pop(gp)
                        for n in range(2):
                            o = ps2[:, n * 512:(n + 1) * 512]
                            w = wr[:, :, n * 512:(n + 1) * 512]
                            nc.tensor.matmul(o, dr_slice(ht, 0, gp), w,
                                             start=True, stop=False,
                                             perf_mode=PM.DoubleRow,
                                             skip_group_check=True)
                            nc.tensor.matmul(o, dr_slice(ht, 1, gp), w,
                                             start=False, stop=False,
                                             perf_mode=PM.DoubleRow,
                                             skip_group_check=True)
                            nc.tensor.matmul(o, ident[:], xc[:, n * 512:(n + 1) * 512],
                                             start=False, stop=(n == 1),
                                             skip_group_check=True)
                        pss[gp] = ps2
                    else:
                        (emit_rproj_neg if m == "C" else emit_rproj)(gp, pss[gp])
                    gslice = gates[:, gp:gp + 1, :]
                    src = pss[gp][:].rearrange("p (g u) -> p g u", u=1024)
                    if m == "C":
                        nc.scalar.activation(gslice, src, AF.Copy,
                                             bias=1.0, scale=-1.0)
                    else:
                        nc.scalar.copy(gslice, src)
                    fixw = int(os.environ.get("KFIXW", "1"))
                    fixeng = nc.vector if (kgc == "dve" or str(gp) in kfix.split(",")) else nc.gpsimd
                    if fixw == 1:
                        _clamp(fixeng, gslice, gslice)
                    elif gp % fixw == fixw - 1:
                        wide = gates[:, gp - fixw + 1:gp + 1, :]
                        _clamp(fixeng, wide, wide)
                    del pss[gp]
                kdefts = [int(v) for v in os.environ.get(
                    "KDEFTS", f"{KDEFT},{KDEFT2}").split(",")]
                if g in kdefts and (s - 1) in pending_tail:
                    for _ in range(kdefts.count(g)):
                        if pending_tail[s - 1]:
                            pending_tail[s - 1].pop(0)()
                if g == T_PER_S and (s - 1) in pending_tail:
                    for f in pending_tail.pop(s - 1):
                        f()
                if g == int(os.environ.get("KSTAGE", "99")) and s + 1 < n_super and (s + 1) not in staged:
                    staged[s + 1] = stage_in(s + 1)
                # last super: emit its own tail in parts, each as soon
                # as the needed gate tiles are evicted+fixed; the final
                # part covers a single tile to keep the drain short
                if last and g >= 3 and g % 2 == 1:
                    emit_tail_part((g - 3) // 2 * 2, (g - 1) // 2 * 2)

            if s + 1 < n_super and (s + 1) not in staged:
                staged[s + 1] = stage_in(s + 1)

            if os.environ.get("KDEFER", "1") == "1" and not last:
                bounds = [0] + [int(v) for v in os.environ.get(
                    "KBOUNDS", "4").split(",")] + [T_PER_S]
                pending_tail[s] = [
                    (lambda lo=bounds[i], hi=bounds[i + 1], f=emit_tail_part: f(lo, hi))
                    for i in range(len(bounds) - 1)]
            else:
                emit_tail_part(6 if last else 0, T_PER_S)

        for fs in list(pending_tail.values()):
            for f in fs:
                f()

    nc.compile()
    return nc


def _pack_activation(a_core):
    """[rows, 256] -> [n_super, 128, T*256] transposed+permuted layout."""
    n_super = a_core.shape[0] // SUPER
    v = a_core.reshape(n_super, 128, T_PER_S, 2, 128)
    return np.ascontiguousarray(v.transpose(0, 4, 3, 2, 1)).reshape(
        n_super, 128, T_PER_S * 256)


FP8NP = mybir.dt.np(FP8)


def _pack_act_fp8_hilo(a_core):
    """[rows, 256] f32 -> [n_super, 128, 8, 2, 2, 128] fp8 hi/lo.

    Layout [s][p][t][hl][k][q]: element [d = k*128 + p, row = s*1024+8q+t],
    per-tile contiguous so tiles stream in order.
    """
    n_super = a_core.shape[0] // SUPER
    hi = a_core.astype(FP8NP)
    lo = (a_core - hi.astype(np.float32)).astype(FP8NP)

    def pk(v):
        w = v.reshape(n_super, 128, T_PER_S, 2, 128)  # [s, q, t, k, p]
        return w.transpose(0, 4, 2, 3, 1)             # [s, p, t, k, q]

    return np.ascontiguousarray(np.stack([pk(hi), pk(lo)], axis=3))


def _pack_weight(w):
    """[256, 1024] +-1 -> [128, 2, 1024] fp8 [p, k, col]."""
    v = w.reshape(2, 128, 1024).transpose(1, 0, 2)
    return np.ascontiguousarray(v.astype(FP8NP))


_PROGRAM_CACHE = {}


def _get_program():
    key = (N_SUPER, GATE_DT)
    if key not in _PROGRAM_CACHE:
        _PROGRAM_CACHE[key] = build_program()
    return _PROGRAM_CACHE[key]


def _run(inputs, h, c, kernel_w, recurrent_kernel, trace=False):
    X = np.ascontiguousarray(np.asarray(inputs, dtype=np.float32))
    H = np.ascontiguousarray(np.asarray(h, dtype=np.float32))
    C = np.ascontiguousarray(np.asarray(c, dtype=np.float32))
    Wk = np.asarray(kernel_w, dtype=np.float32)
    Rk = np.asarray(recurrent_kernel, dtype=np.float32)

    Wb = np.where(Wk >= 0, np.float32(1.0), np.float32(-1.0))
    Rb = np.where(Rk >= 0, np.float32(1.0), np.float32(-1.0))
    # reorder r columns to [r_f, r_i, r_c, r_o] so PSUM accumulation is
    # gate-aligned (f pairs x_i with W_f, i pairs x_f with W_i)
    Rb = np.concatenate(
        [Rb[:, U:2 * U], Rb[:, 0:U], Rb[:, 2 * U:3 * U], Rb[:, 3 * U:]], axis=1)

    wx_np = _pack_weight(Wb)
    wr_np = _pack_weight(Rb)
    wrn_np = _pack_weight(-Rb)
    id_np = np.ascontiguousarray(np.eye(128, dtype=mybir.dt.np(BF16)))

    in_maps = []
    for m in range(N_CORES):
        lo, hi = m * B_CORE, (m + 1) * B_CORE
        in_maps.append({
            "x": _pack_act_fp8_hilo(X[lo:hi]),
            "h": _pack_act_fp8_hilo(np.clip(H[lo:hi], -1.0, 1.0)),
            "c": np.ascontiguousarray(np.clip(C[lo:hi], -1.0, 1.0).astype(
                mybir.dt.np(BF16))).reshape(N_SUPER, 128, 2048),
            "wx": wx_np,
            "wr": wr_np,
            "wrn": wrn_np,
            "ident": id_np,
        })

    nc = _get_program()
    res = run_bass_kernel_spmd(nc, in_maps, core_ids=list(range(N_CORES)),
                               trace=trace)

    h_new = np.empty((B, U), dtype=np.float32)
    c_new = np.empty((B, U), dtype=np.float32)
    for m in range(N_CORES):
        lo, hi = m * B_CORE, (m + 1) * B_CORE
        h_new[lo:hi] = np.asarray(res.results[m]["hn"], dtype=np.float32).reshape(B_CORE, U)
        c_new[lo:hi] = np.asarray(res.results[m]["cn"], dtype=np.float32).reshape(B_CORE, U)
    return (h_new, h_new, c_new), res


def kernel(inputs, h, c, kernel, recurrent_kernel):
    outs, _ = _run(inputs, h, c, kernel, recurrent_kernel, trace=False)
    return outs
